# revision 100
# baseline (speedup 1.0000x reference)
"""MultiHeadAttentionBlock (B=2, S=2048, D=1024, H=16, causal) on 8 trn2 cores.

Sharding: 2-D — batch x head-group. Core c owns batch c//4 and heads
4*(c%4)..+4 (CDIM=256 dims). Each core loads only ITS batch's q/k/v
(12MB vs 24MB for pure head-parallel), projects Q/K/V for its 4 heads over
the full 2048 rows of its batch, and attention is fully local (no KV
exchange). The context is redistributed with TWO global 8-core AllToAlls
(one per head-pair): chunk j carries the sender's pair-dims for rows
[256j, 256j+256) of the sender's batch, so core j ends up owning rows
[256j..+256) of BOTH batches (srcs 0-3 supply batch 0, srcs 4-7 batch 1)
and every (src, dst) chunk is non-empty — all addressing SPMD-uniform and
mesh-legal. Pair 0's A2A fires mid-kernel; its half of every o-proj unit
is pre-accumulated into SBUF f32 partials, so after the final (pair 1)
A2A only 4 accumulation matmuls + a fused (bias + partial) evict remain
per unit. Host only slices / transposes / casts inputs and reassembles
the output (bf16 on device, upcast on host).

Per-core dataflow (matmuls bf16, fp32 PSUM):
  QT/KT[hl] = (w[128 dims] x_b)^T [128, 2048] (dims on partitions; bias on
  DVE evict). V is projected in NATURAL layout (stationary = xT row-block,
  streaming = w_v) straight into per-jt [128 rows, 256 dims] tiles — no PE
  transposes; b_v is folded into b_o on the host (softmax rows sum to 1,
  so ctx bias passes through: b_o' = b_o + w_o @ b_v).
  Attention per (head h, i-half ih): S^T[j,i] = K_j^T Q_i causal blocks
  only; exp on ScalarE (scale=1/8, no max-subtraction needed); triangular
  mask on the diagonal block via VectorE. PV is TRANSPOSED (ex stationary,
  [V_h | 1] streams) so ctx accumulates as [i-rows, dk+1] with the softmax
  denominator per-partition; normalize is a [128,1] reciprocal +
  tensor_scalar multiply. PV is emitted one jt late (software pipeline) so
  its waiters never clog the 4-deep PE wait queue. PSUM allows one
  accumulation group per 2KB bank: 4 i-block slices per ctx tile share one
  start/stop group, normalize batches at group close; normalized blocks
  are PE-transposed to [dims, rows] and staged into a [128, 1024] send
  tile; when BOTH heads of a pair have closed a group, the two dest-core
  chunks ([128, 256], 512B runs — full DMA rate) are sent. o-proj is
  transposed (stationary = w_o^T block, per-partition bias on evict),
  output stored as [odim, rows] and untransposed on the host. x loads are
  1MB DMAs issued ahead in deadline order; proj/vproj/oproj units are PE
  filler inside the exp-paced attention phases. A paced PE trickle spans
  the final collective so the last o-proj runs at the warm p-state.
"""

import os
import numpy as np
import ml_dtypes

B, S, D = 2, 2048, 1024
H, DK = 16, 64
NCORES = 8
GSIZE = 4  # cores per batch group
HPC = 4  # heads per core
CDIM = HPC * DK  # 256 context dims per core
RPC = 512  # output rows per core (2 i-halves x 256)
CPQ = 256  # rows per core per i-half

BF16 = ml_dtypes.bfloat16

_CACHE = {}
LAST_RESULTS = None  # stashed BassKernelResults for external inspection


def _build_program(with_collective=True):
    import concourse.mybir as mybir
    import concourse.tile as tile
    from concourse import bacc
    from concourse.masks import make_identity

    f32 = mybir.dt.float32
    bf = mybir.dt.bfloat16
    Exp = mybir.ActivationFunctionType.Exp

    nc = bacc.Bacc(
        "TRN2", target_bir_lowering=False, debug=False, num_devices=NCORES
    )

    # --- per-core DRAM I/O ---
    xqT_d = nc.dram_tensor("xqT", [D, S], bf, kind="ExternalInput").ap()
    xkT_d = nc.dram_tensor("xkT", [D, S], bf, kind="ExternalInput").ap()
    xvT_d = nc.dram_tensor("xvT", [D, S], bf, kind="ExternalInput").ap()
    wqT_d = nc.dram_tensor("wqT", [128, 8 * CDIM], bf, kind="ExternalInput").ap()
    wkT_d = nc.dram_tensor("wkT", [128, 8 * CDIM], bf, kind="ExternalInput").ap()
    wvT_d = nc.dram_tensor("wvT", [128, 8 * CDIM], bf, kind="ExternalInput").ap()
    bq_d = nc.dram_tensor("bq", [128, 2], f32, kind="ExternalInput").ap()
    bk_d = nc.dram_tensor("bk", [128, 2], f32, kind="ExternalInput").ap()
    woT_d = nc.dram_tensor("woT", [D, D], bf, kind="ExternalInput").ap()
    bo_d = nc.dram_tensor("bo", [128, 8], f32, kind="ExternalInput").ap()
    triu_d = nc.dram_tensor("triu", [128, 128], bf, kind="ExternalInput").ap()
    # out[batch-slot, odim, oblk, row] = transposed 256-row output chunks
    out_d = nc.dram_tensor("out", [2, 128, 8, CPQ], bf, kind="ExternalOutput").ap()

    xd = {"q": xqT_d, "k": xkT_d, "v": xvT_d}

    with tile.TileContext(nc) as tc:
        with (
            tc.tile_pool(name="sb", bufs=1) as sb,
            tc.tile_pool(name="ps", bufs=1, space="PSUM") as ps,
            tc.tile_pool(name="dram", bufs=1, space="DRAM") as dram,
        ):
            # --- weights / consts (issued in need order) ---
            wq3 = sb.tile([128, 8, CDIM], bf, tag="w", bufs=3)
            bq_sb = sb.tile([128, 2], f32, tag="bias", bufs=2)
            bk_sb = sb.tile([128, 2], f32, tag="bias", bufs=2)
            wk3 = sb.tile([128, 8, CDIM], bf, tag="w", bufs=3)
            wv3 = sb.tile([128, 8, CDIM], bf, tag="w", bufs=3)
            bo_sb = sb.tile([128, 8], f32, tag="bo", bufs=1)
            triu_sb = sb.tile([128, 128], bf, tag="triu", bufs=1)
            ones_sb = sb.tile([1, 128], bf, tag="ones", bufs=1)
            nc.vector.memset(ones_sb, 1.0)
            ident_sb = sb.tile([128, 128], bf, tag="ident", bufs=1)
            make_identity(nc, ident_sb)
            # preload the exp table set during the DMA ramp so the first real
            # exp doesn't pay the ~1.3us ACT_TABLE_LOAD
            warm_sb = sb.tile([128, 128], bf, tag="warm", bufs=1)
            nc.vector.memset(warm_sb, 1.0)
            nc.scalar.activation(
                out=warm_sb[0:1, 0:1], in_=ones_sb[0:1, 0:1], func=Exp, scale=1.0
            )
            warm2_sb = sb.tile([1, 512], bf, tag="warm2", bufs=1)
            nc.vector.memset(warm2_sb, 1.0)
            wo3 = sb.tile([128, 8, D], bf, tag="wo", bufs=1)

            # send/recv DRAM for the 2 global 8-core AllToAlls (one per
            # head-pair). Chunk j of each A2A carries the sender's ctx dims
            # for the 256 rows (of the sender's batch) that core j owns —
            # every (src, dst) chunk is non-empty, so a plain mesh AllToAll
            # over all 8 cores works and all addressing is SPMD-uniform.
            send_q = {}
            recv_q = {}
            for p in range(2):
                send_q[p] = dram.tile(
                    [NCORES, 128, CPQ], bf, tag=f"snd{p}", name=f"send{p}"
                )
                recv_q[p] = dram.tile(
                    [NCORES, 128, CPQ], bf, tag=f"rcv{p}", name=f"recv{p}"
                )

            # persistent tiles: QT/KT per head-pair, V3 natural [j, head|1]
            QT = {hl: sb.tile([128, S], bf, tag="qt", bufs=2, name=f"QT{hl}")
                  for hl in range(2)}
            KT = {hl: sb.tile([128, S], bf, tag="kt", bufs=2, name=f"KT{hl}")
                  for hl in range(2)}
            V3 = sb.tile([128, 16, 4 * 65], bf, tag="v3", bufs=1, name="V3")
            for h in range(HPC):
                nc.vector.memset(V3[:, :, 65 * h + 64 : 65 * h + 65], 1.0)

            # send staging per pair: [dim 128, global row 2048]
            ST = {
                p: sb.tile([128, 2 * S // 2], bf, tag="st", bufs=2,
                           name=f"st{p}")
                for p in range(2)
            }
            # SBUF f32 partials: pair-0's half of each o-proj unit is
            # accumulated mid-kernel (its A2A lands early) so the tail only
            # runs the pair-1 half
            PP = {
                bp: sb.tile([128, 8, CPQ], f32, tag="pp", bufs=2,
                            name=f"pp{bp}")
                for bp in range(2)
            }

            xts = {}  # (pref, n) -> loaded [128, 8, 512] chunk

            def load_xc(pref, n, split=1):
                # 1MB for rows [512n, 512n+512) (1KB-contig runs); split>1
                # issues sub-DMAs along the contraction dim so the first
                # proj matmuls start before the full chunk lands
                t = sb.tile(
                    [128, 8, 512], bf, tag="xt", bufs=8, name=f"x{pref}{n}"
                )
                co = 512 * n
                kstep = 8 // split
                for g in range(split):
                    nc.sync.dma_start(
                        out=t[:, g * kstep : (g + 1) * kstep, :],
                        in_=xd[pref][
                            128 * kstep * g : 128 * kstep * (g + 1), co : co + 512
                        ].rearrange("(ko ki) m -> ki ko m", ki=128),
                    )
                xts[(pref, n)] = t

            W3 = {"q": wq3, "k": wk3}
            BS = {"q": bq_sb, "k": bk_sb}
            OT = {"q": QT, "k": KT}

            def proj(pref, hl, n, quarter=None):
                # Q/K projection for head-pair hl. quarter=0..3 does a
                # 128-row sub-chunk (~427ns of PE) whose PSUM opens AND
                # closes within the call, sized so a filler slot never
                # stalls the exp stream.
                xt = xts[(pref, n)]
                qs = range(4) if quarter is None else [quarter]
                for q in qs:
                    cs = slice(128 * q, 128 * q + 128)
                    pt = ps.tile([128, 128], f32, tag="proj", bufs=2,
                                 name="pproj")
                    for kk in range(8):
                        nc.tensor.matmul(
                            pt,
                            W3[pref][:, kk, 128 * hl : 128 * hl + 128],
                            xt[:, kk, cs],
                            start=(kk == 0),
                            stop=(kk == 7),
                        )
                    nc.vector.tensor_scalar_add(
                        out=OT[pref][hl][
                            :, n * 512 + 128 * q : n * 512 + 128 * q + 128
                        ],
                        in0=pt,
                        scalar1=BS[pref][:, hl : hl + 1],
                    )

            def vproj(rt, dh=None):
                # V rows [128rt, 128rt+128) in natural [rows, dims] layout:
                # stationary = xvT row-block, streaming = w_v. dh=0/1 does
                # one head-pair's 128 dims (self-closing ~427ns unit). Copy
                # per head into V3 (ones cols preset).
                xt = xts[("v", rt // 4)]
                cs = slice(128 * (rt % 4), 128 * (rt % 4) + 128)
                dhs = range(2) if dh is None else [dh]
                for d in dhs:
                    pv = ps.tile([128, 128], f32, tag="proj", bufs=2,
                                 name="pvn")
                    for kk in range(8):
                        nc.tensor.matmul(
                            pv, xt[:, kk, cs],
                            wv3[:, kk, 128 * d : 128 * d + 128],
                            start=(kk == 0), stop=(kk == 7),
                        )
                    for h in (2 * d, 2 * d + 1):
                        nc.vector.tensor_copy(
                            out=V3[:, rt, 65 * h : 65 * h + 64],
                            in_=pv[:, 64 * h - 128 * d : 64 * h - 128 * d + 64],
                        )

            rcvt = {}

            def rcv(p, eng=None):
                t = sb.tile([128, NCORES, CPQ], bf, tag="rcvt", bufs=2,
                            name=f"rc{p}")
                (eng or nc.sync).dma_start(
                    out=t, in_=recv_q[p].rearrange("s d r -> d s r")
                )
                rcvt[p] = t

            def pacc(bp, oblk, tag="proj"):
                # pair-0 half of o-proj unit (bp, oblk), evicted to the f32
                # partial tile (pair 0's A2A lands mid-kernel)
                po = ps.tile([128, CPQ], f32, tag=tag, bufs=2,
                             name=f"pa{bp}{oblk}")
                cs = slice(128 * oblk, 128 * oblk + 128)
                for s in range(GSIZE):
                    src = 4 * bp + s
                    nc.tensor.matmul(
                        po, wo3[:, 2 * s, cs], rcvt[0][:, src, :],
                        start=(s == 0), stop=(s == GSIZE - 1),
                    )
                nc.vector.tensor_copy(out=PP[bp][:, oblk, :], in_=po)

            def oproj(bp, oblk, ob3, tag="proj"):
                # pair-1 half + fused (bias + partial) evict into the store
                # staging tile
                po = ps.tile([128, CPQ], f32, tag=tag, bufs=2,
                             name=f"po{bp}{oblk}")
                cs = slice(128 * oblk, 128 * oblk + 128)
                for s in range(GSIZE):
                    src = 4 * bp + s
                    nc.tensor.matmul(
                        po, wo3[:, 2 * s + 1, cs], rcvt[1][:, src, :],
                        start=(s == 0), stop=(s == GSIZE - 1),
                    )
                nc.vector.scalar_tensor_tensor(
                    out=ob3[:, oblk, :], in0=po,
                    scalar=bo_sb[:, oblk : oblk + 1],
                    in1=PP[bp][:, oblk, :],
                    op0=mybir.AluOpType.add, op1=mybir.AluOpType.add,
                )

            def send_half(ih, p, g, eng=None):
                # a closed group's 4 i-blocks = the chunks of 2 dest cores
                for u in range(2):
                    dest = 2 * (2 * ih + g) + u
                    (eng or nc.gpsimd).dma_start(
                        out=send_q[p][dest],
                        in_=ST[p][
                            :, 1024 * ih + 512 * g + 256 * u :
                            1024 * ih + 512 * g + 256 * u + 256
                        ],
                    )

            def attention(h, ih, fillers, carry=()):
                # fillers: jt -> list of thunks (later-phase PE work) injected
                # so TensorE stays fed while ScalarE paces the exp stream.
                # carry: the previous phase's tail work (last PV group, final
                # close, its pair's a2a) — emitted after this phase's first
                # scores/exp so the exp stream never drains at boundaries.
                hl, hp = h // 2, h % 2
                pb = 64 * hp
                ibase = 1024 * ih
                ctx = {}

                def get_ctx(g):
                    # lazy: allocated at first PV emission, AFTER the carry,
                    # so the PSUM ring stays acyclic across phases
                    if g not in ctx:
                        ctx[g] = ps.tile([128, 4, 65], f32, tag="ctx", bufs=2,
                                         name=f"ctx{h}{ih}{g}")
                    return ctx[g]

                pend_close = []

                def pv_batch(jt, ex, base):
                    # PV per i-block: ctx[ib] += ex_blk^T @ [V_h | 1].
                    # Emitted one group late (software pipeline). The 4
                    # i-block slices in a ctx tile share one PSUM
                    # accumulation group. The diagonal i-block (masked on
                    # DVE after the exp) goes LAST so its mask latency hides
                    # behind the other blocks' matmuls — except at jt==0,
                    # whose first matmul must carry the group-start flag.
                    ibls = list(range(max(0, jt - 8 * ih), 8))
                    if jt // 8 == ih and jt > 0 and len(ibls) > 1:
                        ibls = ibls[1:] + ibls[:1]
                    for ibl in ibls:
                        nc.tensor.matmul(
                            get_ctx(ibl // 4)[:, ibl % 4, :],
                            ex[:, base + 128 * ibl : base + 128 * ibl + 128],
                            V3[:, jt, 65 * h : 65 * h + 65],
                            start=(jt == 0 and ibl % 4 == 0),
                            stop=(jt == ibl + 8 * ih and ibl % 4 == 3),
                        )
                    gdone = jt - 8 * ih
                    if 0 <= gdone < 8 and gdone % 4 == 3:
                        pend_close.append(gdone // 4)

                def close_stage1(g):
                    # normalize the closed bank group's 4 i-blocks (PSUM col
                    # 64 = denominator): reciprocal + per-partition multiply.
                    # The PE transposes happen in stage 2 AFTER the filler
                    # slot so they never wait on this DVE chain.
                    cns = []
                    for ibl in range(4 * g, 4 * g + 4):
                        cx = get_ctx(g)[:, ibl % 4, :]
                        rs = sb.tile([128, 1], f32, tag="rs", bufs=8, name="rs")
                        nc.vector.reciprocal(out=rs, in_=cx[:, 64:65])
                        cn = sb.tile([128, 64], bf, tag="cn", bufs=8, name="cn")
                        nc.vector.tensor_scalar_mul(
                            out=cn, in0=cx[:, 0:64], scalar1=rs
                        )
                        cns.append(cn)
                    return cns

                def close_stage2(g, cns):
                    # 4 transposes into one [64, 512] PSUM tile, ONE staging
                    # copy, then the group's dest-chunk sends if the pair is
                    # complete
                    ct4 = ps.tile([64, 512], bf, tag="ctx", bufs=2, name="ct4")
                    for j, cn in enumerate(cns):
                        nc.tensor.transpose(
                            ct4[:, 128 * j : 128 * j + 128], cn, ident_sb
                        )
                    nc.vector.tensor_copy(
                        out=ST[hl][
                            pb : pb + 64,
                            1024 * ih + 512 * g : 1024 * ih + 512 * g + 512,
                        ],
                        in_=ct4,
                    )
                    if hp == 1:
                        # each pair's final sends ride the idle SP queue so
                        # its AllToAll isn't gated on serial SWDGE
                        # descriptor generation
                        tailg = ih == 1 and g == 1
                        send_half(ih, hl, g, eng=nc.sync if tailg else None)

                def pv_group(grp, ex, exoff):
                    for jt in grp:
                        pv_batch(jt, ex, exoff[jt])

                # one jt per exp: PSUM is fp32-only for matmul output on
                # TRN2, and a [128, 2048] f32 pair tile would blow the PSUM
                # bank budget.
                njt = 8 * (ih + 1)
                groups = [(jt,) for jt in range(njt)]

                pend = None  # (grp, ex, exoff) not yet emitted (one late)
                carried = False
                for grp in groups:
                    ew = 1024 * len(grp)
                    ex = sb.tile([128, ew], bf, tag="ex", bufs=8, name="ex")
                    sc = ps.tile([128, ew], f32, tag="sc", bufs=2, name="sc")
                    # pair layout: [odd jt at 0:1024 | even jt at 1024:2048]
                    exoff = {jt: 0 for jt in grp}
                    if len(grp) == 2:
                        exoff[grp[0]] = 1024
                    lo = 2048
                    for jt in grp:
                        jpos = 128 * jt
                        off0 = max(jpos, ibase) - ibase
                        base = exoff[jt]
                        lo = min(lo, base + off0)
                        off = off0
                        first_chunk = True
                        while off < 1024:
                            cw = min(512 - off % 512, 1024 - off)
                            nc.tensor.matmul(
                                sc[:, base + off : base + off + cw],
                                KT[hl][pb : pb + 64, jpos : jpos + 128],
                                QT[hl][
                                    pb : pb + 64, ibase + off : ibase + off + cw
                                ],
                                start=True,
                                stop=True,
                            )
                            off += cw
                            first_chunk = False
                            if h == 0 and ih == 0 and jt == 0:
                                nc.scalar.activation(
                                    out=ex[:, off - cw : off],
                                    in_=sc[:, off - cw : off],
                                    func=Exp,
                                    scale=0.125,
                                )
                    if not (h == 0 and ih == 0 and grp[0] == 0):
                        nc.scalar.activation(
                            out=ex[:, lo:ew], in_=sc[:, lo:ew],
                            func=Exp, scale=0.125,
                        )
                    for jt in grp:
                        if jt // 8 == ih:
                            # diagonal block lives in this i-half: mask it
                            # after the exp (the PV for this block is
                            # reordered last, hiding the DVE latency)
                            dg = 128 * jt - ibase + exoff[jt]
                            nc.vector.tensor_mul(
                                ex[:, dg : dg + 128], ex[:, dg : dg + 128],
                                triu_sb,
                            )
                    # carry[0] (prev phase's last PV + close divides) comes
                    # before this group's fillers; carry[1:] (the close
                    # transposes/sends and the pair's a2a) after them, so
                    # the divide->transpose chain is hidden behind fillers
                    pre = []
                    if not carried:
                        pre = list(carry)
                        if pre:
                            pre[0]()
                        carried = True
                    # deferred close normalize (stage 1: DVE divides), then
                    # fillers AFTER the scores/exp so they absorb the
                    # exp-wait instead of delaying the exp stream, then the
                    # close transposes (their divides are done by now)
                    closes = [(g, close_stage1(g)) for g in pend_close]
                    pend_close.clear()
                    for jt in grp:
                        for f in fillers.get(jt, ()):
                            f()
                    for t in pre[1:]:
                        t()
                    for g, cns in closes:
                        close_stage2(g, cns)
                    if pend is not None:
                        pv_group(*pend)
                    pend = (grp, ex, exoff)

                def tail_pv(pend=pend):
                    pv_group(*pend)
                    tail_cns.extend(
                        (g, close_stage1(g)) for g in pend_close
                    )
                    pend_close.clear()

                tail_cns = []

                def tail_close():
                    for g, cns in tail_cns:
                        close_stage2(g, cns)
                return [tail_pv, tail_close]

            def a2a(p):
                if with_collective:
                    nc.gpsimd.collective_compute(
                        "AllToAll",
                        mybir.AluOpType.bypass,
                        replica_groups=[list(range(NCORES))],
                        ins=[send_q[p].opt()],
                        outs=[recv_q[p].opt()],
                    )
                else:
                    # timing-only stand-in (TimelineSim has no collectives);
                    # SP queue: the Tile scheduler has been observed to
                    # defer Pool-queued transfers tens of us past readiness
                    nc.sync.dma_start(out=recv_q[p], in_=send_q[p])

            L = lambda pref, n, split=1: (lambda: load_xc(pref, n, split))
            PQ = lambda pref, hl, n, q: (lambda: proj(pref, hl, n, q))
            VP = lambda rt, dh: (lambda: vproj(rt, dh))
            PA = lambda bp, oblk: (lambda: pacc(bp, oblk))

            def Lwo():
                nc.sync.dma_start(
                    out=wo3, in_=woT_d.rearrange("(ko ki) m -> ki ko m", ki=128)
                )

            def sched(*pairs):
                d = {}
                for jt, t in pairs:
                    d.setdefault(jt, []).append(t)
                return d

            # --- prologue: minimal data for attention(0, 0); load stream in
            # global consumption-deadline order. Tiny bias/const loads ride
            # the ACT queue so they don't delay the x-chunk stream. ---
            nc.sync.dma_start(out=wk3, in_=wkT_d)
            load_xc("k", 0, split=4)
            nc.scalar.dma_start(out=bk_sb, in_=bk_d)
            nc.scalar.dma_start(out=bq_sb, in_=bq_d)
            nc.sync.dma_start(out=wq3, in_=wqT_d)
            load_xc("q", 0, split=4)
            proj("k", 0, 0)
            load_xc("q", 1, split=2)
            proj("q", 0, 0)
            proj("q", 0, 1)
            nc.sync.dma_start(out=wv3, in_=wvT_d)
            load_xc("v", 0, split=2)
            nc.scalar.dma_start(out=triu_sb, in_=triu_d)
            vproj(0)
            load_xc("k", 1, split=2)
            load_xc("v", 1, split=2)
            nc.scalar.dma_start(out=bo_sb, in_=bo_d)

            # --- phases; fillers are ~427ns self-closing units spread at
            # every jt so the exp stream never stalls and PE never starves.
            # Deadlines: VP(1..7)+P(k,0,1) forced into the first phase;
            # KT[hl] ch2/ch3 before (2hl,1) jt8/jt12; QT[hl] ch2+ch3 before
            # (2hl,1) jt0; V3 rt before (0,1)'s jt rt+1 ---
            cr = attention(0, 0, sched(
                (0, VP(1, 0)), (0, VP(1, 1)), (1, VP(2, 0)), (1, VP(2, 1)),
                (2, VP(3, 0)), (2, VP(3, 1)),
                (3, PQ("k", 0, 1, 0)), (3, PQ("k", 0, 1, 1)),
                (4, PQ("k", 0, 1, 2)), (4, PQ("k", 0, 1, 3)),
                (4, VP(4, 0)), (5, VP(4, 1)), (5, VP(5, 0)), (5, VP(5, 1)),
                (6, VP(6, 0)), (6, VP(6, 1)), (7, VP(7, 0)), (7, VP(7, 1)),
                (1, L("q", 2)), (6, L("v", 2)),
            ))
            cr = attention(1, 0, sched(
                (0, PQ("k", 1, 0, 0)), (0, PQ("k", 1, 0, 1)),
                (1, PQ("k", 1, 0, 2)), (1, PQ("k", 1, 0, 3)),
                (2, PQ("k", 1, 1, 0)), (2, PQ("k", 1, 1, 1)),
                (3, PQ("k", 1, 1, 2)), (3, PQ("k", 1, 1, 3)),
                (4, PQ("q", 1, 0, 0)), (4, PQ("q", 1, 0, 1)),
                (5, PQ("q", 1, 0, 2)), (5, PQ("q", 1, 0, 3)),
                (6, PQ("q", 1, 1, 0)), (6, PQ("q", 1, 1, 1)),
                (7, PQ("q", 1, 1, 2)), (7, PQ("q", 1, 1, 3)),
                (5, L("k", 2)), (7, L("q", 3)),
            ), carry=cr)
            cr = attention(2, 0, sched(
                (0, L("v", 3)), (0, PQ("q", 0, 2, 0)), (1, PQ("q", 0, 2, 1)),
                (2, PQ("q", 0, 2, 2)), (3, PQ("q", 0, 2, 3)),
                (4, VP(8, 0)), (5, VP(8, 1)), (5, L("k", 3)),
                (6, VP(9, 0)), (7, VP(9, 1)),
            ), carry=cr)
            cr = attention(3, 0, sched(
                (0, PQ("q", 0, 3, 0)), (1, PQ("q", 0, 3, 1)),
                (2, PQ("q", 0, 3, 2)), (3, PQ("q", 0, 3, 3)),
                (4, Lwo), (4, VP(10, 0)), (5, VP(10, 1)),
                (6, VP(11, 0)), (6, VP(11, 1)), (7, VP(12, 0)),
                (7, VP(12, 1)),
            ), carry=cr)
            cr = attention(0, 1, sched(
                (0, PQ("k", 0, 2, 0)), (1, PQ("k", 0, 2, 1)),
                (2, PQ("k", 0, 2, 2)), (3, PQ("k", 0, 2, 3)),
                (4, PQ("k", 0, 3, 0)), (5, PQ("k", 0, 3, 1)),
                (6, PQ("k", 0, 3, 2)), (7, PQ("k", 0, 3, 3)),
                (8, PQ("q", 1, 2, 0)), (9, PQ("q", 1, 2, 1)),
                (10, VP(13, 0)), (11, VP(13, 1)),
                (12, VP(14, 0)), (13, VP(14, 1)),
                (14, VP(15, 0)), (15, VP(15, 1)),
            ), carry=cr)
            cr = attention(1, 1, sched(
                (0, PQ("q", 1, 2, 2)), (1, PQ("q", 1, 2, 3)),
                (2, PQ("q", 1, 3, 0)), (3, PQ("q", 1, 3, 1)),
                (4, PQ("q", 1, 3, 2)), (5, PQ("q", 1, 3, 3)),
                (6, PQ("k", 1, 2, 0)), (7, PQ("k", 1, 2, 1)),
                (8, PQ("k", 1, 2, 2)), (9, PQ("k", 1, 2, 3)),
            ), carry=cr)
            def _a2a0():
                # high_priority: nudge the Tile scheduler to place the
                # pair-0 collective + receive at their readiness time
                # instead of deferring them behind later Pool/SP work
                with tc.high_priority(offset=2000):
                    a2a(0)
                    rcv(0)
            cr.append(_a2a0)
            cr = attention(2, 1, sched(
                (4, PQ("k", 1, 3, 0)), (5, PQ("k", 1, 3, 1)),
                (6, PQ("k", 1, 3, 2)), (7, PQ("k", 1, 3, 3)),
            ), carry=cr)
            cr = attention(3, 1, sched(), carry=cr)
            # final phase's tail: last PV + divides, transposes/sends, then
            # the pair-0 pre-accumulation batch — real PE work that covers
            # the final collective's latency and keeps the p-state warm
            cr[0]()
            for t in cr[1:]:
                t()
            a2a(1)
            rcv(1, eng=nc.scalar)
            # continuous trickle holds the p-state through the
            # scheduler-deferred rcv(0). It reads the LAST-written staging
            # region so the scheduler cannot hoist it into mid-phase holes
            # (a dep-free trickle gets scheduled wherever PE idles).
            for _ in range(25):
                pwk = ps.tile([128, 512], f32, tag="proj", bufs=2, name="pwk")
                nc.tensor.matmul(
                    pwk, ST[1][64:65, 1536:1664], ST[1][64:65, 1536:2048],
                    start=True, stop=True,
                )
            # round-robin the tail units across the proj/sc/ctx PSUM rings
            # (sc/ctx are idle post-phases): a 6-deep ring means no unit's
            # matmuls ever wait on a DVE evict two units back
            TT = ["proj", "proj", "sc", "sc", "ctx", "ctx"]
            for bp in range(2):
                for oblk in range(8):
                    pacc(bp, oblk, tag=TT[(8 * bp + oblk) % 6])
            # final o-proj: pair-1 halves + fused (bias + pair-0 partial)
            # evicts; stores batched across SP/ACT queues
            ob3 = {
                bp: sb.tile([128, 8, CPQ], bf, tag="ob3", bufs=2,
                            name=f"ob3_{bp}")
                for bp in range(2)
            }
            for oblk in range(4):
                oproj(0, oblk, ob3[0], tag=TT[oblk % 6])
            nc.sync.dma_start(out=out_d[0][:, 0:4, :], in_=ob3[0][:, 0:4, :])
            for oblk in range(4):
                oproj(1, oblk, ob3[1], tag=TT[(4 + oblk) % 6])
            nc.sync.dma_start(out=out_d[1][:, 0:4, :], in_=ob3[1][:, 0:4, :])
            for oblk in range(4, 8):
                oproj(0, oblk, ob3[0], tag=TT[(8 + oblk - 4) % 6])
            nc.sync.dma_start(out=out_d[0][:, 4:8, :], in_=ob3[0][:, 4:8, :])
            for oblk in range(4, 6):
                oproj(1, oblk, ob3[1], tag=TT[(12 + oblk - 4) % 6])
            nc.sync.dma_start(out=out_d[1][:, 4:6, :], in_=ob3[1][:, 4:6, :])
            # last two stores on different DGE paths so their fixed
            # HWDGE/dge/sem latencies overlap instead of serializing
            oproj(1, 6, ob3[1], tag=TT[14 % 6])
            nc.scalar.dma_start(out=out_d[1][:, 6:7, :], in_=ob3[1][:, 6:7, :])
            oproj(1, 7, ob3[1], tag=TT[15 % 6])
            nc.sync.dma_start(out=out_d[1][:, 7:8, :], in_=ob3[1][:, 7:8, :])

    nc.compile()
    return nc


def _prep_inputs(q, k, v, w_q, b_q, w_k, b_k, w_v, b_v, w_o, b_o):
    def bf(x):
        return np.ascontiguousarray(x).astype(BF16)

    def prew(w, m):
        # host-side (ko ki) m -> ki ko m rearrange so the device weight DMA
        # has 2KB-contiguous runs (full DMA bandwidth)
        return bf(w.reshape(8, 128, m).transpose(1, 0, 2).reshape(128, 8 * m))

    q = np.asarray(q, np.float32)
    k = np.asarray(k, np.float32)
    v = np.asarray(v, np.float32)
    w_q = np.asarray(w_q, np.float32)
    w_k = np.asarray(w_k, np.float32)
    w_v = np.asarray(w_v, np.float32)
    w_o = np.asarray(w_o, np.float32)
    b_q = np.asarray(b_q, np.float32)
    b_k = np.asarray(b_k, np.float32)
    b_v = np.asarray(b_v, np.float32)
    b_o = np.asarray(b_o, np.float32)

    xT = {pref: [bf(x[b].T) for b in range(B)] for pref, x in
          (("q", q), ("k", k), ("v", v))}
    woT = bf(w_o.T)  # [1024 ctx-dims head-major, 1024 odims]; dev rearranges
    bo_eff = b_o + w_o @ b_v  # fold V bias through softmax into o bias
    bo_host = np.ascontiguousarray(bo_eff.reshape(8, 128).T)  # [128, 8]
    triu = np.triu(np.ones((128, 128), np.float32)).astype(BF16)

    in_maps = []
    for c in range(NCORES):
        b, ct = c // GSIZE, c % GSIZE
        hs = slice(ct * CDIM, (ct + 1) * CDIM)
        in_maps.append(
            {
                "xqT": xT["q"][b],
                "xkT": xT["k"][b],
                "xvT": xT["v"][b],
                "wqT": prew(w_q[hs, :].T, CDIM),
                "wkT": prew(w_k[hs, :].T, CDIM),
                "wvT": prew(w_v[hs, :].T, CDIM),
                "bq": np.ascontiguousarray(b_q[hs].reshape(2, 128).T),
                "bk": np.ascontiguousarray(b_k[hs].reshape(2, 128).T),
                "woT": woT,
                "bo": bo_host,
                "triu": triu,
            }
        )
    return in_maps


def kernel(q, k, v, mask, w_q, b_q, w_k, b_k, w_v, b_v, w_o, b_o):
    global LAST_RESULTS
    if "nc" not in _CACHE:
        _CACHE["nc"] = _build_program()
    nc = _CACHE["nc"]

    from concourse.bass_utils import run_bass_kernel_spmd

    in_maps = _prep_inputs(q, k, v, w_q, b_q, w_k, b_k, w_v, b_v, w_o, b_o)
    res = run_bass_kernel_spmd(nc, in_maps, core_ids=list(range(NCORES)))
    LAST_RESULTS = res
    # core c = (batch c//4, group slot c%4): out[ih, oblk, odim, r] covers
    # global rows [1024 ih + 256 (c%4), +256), cols [128 oblk, +128)
    # core c owns rows [256c, 256c+256) of BOTH batches (the global A2A's
    # uniform chunk->row mapping)
    out = np.empty((B, S, D), np.float32)
    for c in range(NCORES):
        oc = np.asarray(res.results[c]["out"], np.float32)  # [2, 128, 8, 256]
        for b in range(B):
            r0 = CPQ * c
            out[b, r0 : r0 + CPQ, :] = oc[b].transpose(2, 1, 0).reshape(CPQ, D)
    return out
